# revision 1
# baseline (speedup 1.0000x reference)
"""AngleLossV2 distributed Bass kernel for 8 TRN2 NeuronCores.

Math (reference):
  mask[a,p,q] = pm[a,p] & pm[a,q] & (a!=p) & (a!=q) & (p!=q)
  fn = l2norm(feat, -1); tn = l2norm(true, -1)
  f[a,p,q] = <fn[a,p], fn[a,q]>;  t likewise
  cnt = sum(mask); tp = where(mask, t-eps, 0); s1 = sum(tp); s2 = sum(tp*tp)
  d = sqrt(max(cnt*f^2 - 2*f*s1 + s2, 0))
  loss = 0.5 * sum(where(mask, d, 0)) / max(cnt, 1)

Key algebra (per anchor a, over masked normalized rows z_p):
  sum_{p!=q valid} t   = ||sum_p z_p||^2 - k_a
  sum_{p!=q valid} t^2 = ||Z^T Z||_F^2 - k_a      (Z^T Z is [128,128])
  cnt = sum_a (k_a^2 - k_a), K1 = sum_a k_a       (host, exact)
so s1/s2 need only O(N^2 D) work.  Phase 2 computes the per-anchor Gram
f~ = Zf Zf^T (upper-triangle blocks, off-diag weighted x2 in the reduce):
  d(x) = sqrt(cnt*(x-mu)^2 + c2g),  mu = s1/cnt, c2g = s2 - s1^2/cnt
Invalid/pad entries have x = 0 exactly, valid diagonal x ~= 1:
  sum_valid d = sum_all d - (N*NR^2 - cnt - K1)*d0 - K1*d1
d0/d1 are probed on-chip through the exact same instruction chain (ACT
Square then Sqrt, same dtypes incl. the bf16 rounding of d) so LUT and
rounding bias cancels exactly.

Sparsity: the host compacts each anchor's VALID rows (mask order is
irrelevant to the sums) and zero-pads to NR=256, shrinking the Gram work
~2x and the loaded bytes ~33%.  ZfT is produced by the PE itself:
ZfT_c = Fb_c^T @ diag(winv_c) — transpose, mask and 1/norm fused into one
matmul.  d sums accumulate on the PE via ones/twos-vector matmuls into a
long-lived PSUM accumulation group.  One AllReduce of 2 scalars between
phases.  Host combines per-core partials in float64.
"""

import sys
import numpy as np

for _p in ("/opt/trn_rl_repo",):
    if _p not in sys.path:
        sys.path.insert(0, _p)

from concourse import bacc, bass, mybir, tile
from concourse import bass_utils

F32 = mybir.dt.float32
BF16 = mybir.dt.bfloat16
AF = mybir.ActivationFunctionType
ALU = mybir.AluOpType

N = 384
D = 128
NCORES = 8
SLAB = N // NCORES  # 48 anchors per core
D1 = D + 4  # z chunk + ones column + pad (keeps 4B alignment)
NORM_EPS = 1e-6
PD_EPS = 1e-6

# out row layout ([1, NOUT])
O_DSUM = 0  # weighted d column sums (diag*1 + off*2 accumulated)
O_D0A = 768
O_D1 = 769
O_DBG = 772  # s1,s2,1/cnt,mu,negmu
O_AR = 778  # arin0, arin1, arout0, arout1
NOUT = 784

_CACHE = {}


def _build(NR):
    CH = NR // 128  # row chunks per anchor
    DIAGW = CH * D
    OFFW = (CH * (CH - 1) // 2) * D
    UW = DIAGW + OFFW  # u2/d width per anchor

    nc = bacc.Bacc(
        "TRN2",
        target_bir_lowering=False,
        debug=False,
        num_devices=NCORES,
    )
    tru_t = nc.dram_tensor("tru", [SLAB, NR, D], F32, kind="ExternalInput")
    fea_t = nc.dram_tensor("fea", [SLAB, NR, D], F32, kind="ExternalInput")
    wmk_t = nc.dram_tensor("wmk", [128, SLAB * CH], F32, kind="ExternalInput")
    scl_t = nc.dram_tensor("scl", [1, 2], F32, kind="ExternalInput")
    eye_t = nc.dram_tensor("eye", [128, 128], F32, kind="ExternalInput")
    out_t = nc.dram_tensor("out", [1, NOUT], F32, kind="ExternalOutput")

    tru = tru_t.ap()
    fea = fea_t.ap()
    wmk = wmk_t.ap()
    scl = scl_t.ap()
    eye = eye_t.ap()
    out = out_t.ap()

    PAIR = 2  # anchors per load DMA

    with tile.TileContext(nc) as tc:
        with (
            tc.tile_pool(name="slab", bufs=1) as slab_pool,
            tc.tile_pool(name="stat", bufs=1) as stat,
            tc.tile_pool(name="work", bufs=3) as work,
            tc.tile_pool(name="dram", bufs=1, space="DRAM") as dram,
        ):
            # ---- persistent tiles ----
            slabT = slab_pool.tile([128, SLAB * CH * D], F32, tag="slabT")
            Zt0 = stat.tile([128, CH * D1], BF16, tag="Zt0")
            Zt1 = stat.tile([128, CH * D1], BF16, tag="Zt1")
            n2t = stat.tile([128, SLAB * CH], F32, tag="n2t")
            nrm = stat.tile([128, SLAB * CH], F32, tag="nrm")
            winv = stat.tile([128, SLAB * CH], F32, tag="winv")
            wmks = stat.tile([128, SLAB * CH], F32, tag="wmks")
            sclT = stat.tile([1, 2], F32, tag="sclT")
            outsb = stat.tile([1, NOUT], F32, tag="outsb")
            v2b = stat.tile([128, SLAB], F32, tag="v2b")
            F2b = stat.tile([128, SLAB], F32, tag="F2b")
            onesb = stat.tile([128, 1], BF16, tag="onesb")
            twosb = stat.tile([128, 1], BF16, tag="twosb")
            onesf = stat.tile([128, 1], F32, tag="onesf")
            ones1 = stat.tile([1, 128], F32, tag="ones1")
            eyeb = stat.tile([128, 128], BF16, tag="eyeb")
            eyef = stat.tile([128, 128], F32, tag="eyef")

            nc.vector.memset(onesb[:], 1.0)
            nc.vector.memset(twosb[:], 2.0)
            nc.vector.memset(onesf[:], 1.0)
            nc.vector.memset(ones1[:], 1.0)
            nc.vector.memset(outsb[:], 0.0)
            nc.sync.dma_start(wmks[:], wmk)
            nc.sync.dma_start(sclT[:], scl)
            nc.sync.dma_start(eyef[:], eye)
            nc.vector.tensor_copy(eyeb[:], eyef[:])
            for zt in (Zt0, Zt1):
                for c in range(CH):
                    nc.vector.memset(zt[:, c * D1 + D : (c + 1) * D1], 1.0)

            def aseg(a):
                return slabT[:, a * CH * D : (a + 1) * CH * D]

            def load_slab(src):
                for gi, a0 in enumerate(range(0, SLAB, PAIR)):
                    sl = slabT[:, a0 * CH * D : (a0 + PAIR) * CH * D]
                    eng = nc.sync if gi % 2 == 0 else nc.scalar
                    eng.dma_start(
                        sl.rearrange("p (b c d) -> p b c d", b=PAIR, d=D),
                        src[a0 : a0 + PAIR].rearrange("b (c p) d -> p b c d", p=128),
                    )

            def norms_anchor(a):
                for c in range(CH):
                    scr = work.tile([128, D], BF16, tag="scr")
                    nc.vector.scalar_tensor_tensor(
                        out=scr[:],
                        in0=slabT[:, (a * CH + c) * D : (a * CH + c + 1) * D],
                        scalar=1.0,
                        in1=slabT[:, (a * CH + c) * D : (a * CH + c + 1) * D],
                        op0=ALU.mult,
                        op1=ALU.mult,
                        accum_out=n2t[:, a * CH + c : a * CH + c + 1],
                    )

            def finish_winv():
                nc.scalar.activation(nrm[:], n2t[:], AF.Sqrt)
                nc.vector.tensor_scalar_max(nrm[:], nrm[:], NORM_EPS)
                nc.vector.reciprocal(nrm[:], nrm[:])
                nc.vector.tensor_tensor(winv[:], nrm[:], wmks[:], op=ALU.mult)

            # ================= phase 1: true =================
            load_slab(tru)
            for a in range(SLAB):
                norms_anchor(a)
            finish_winv()

            with tc.tile_pool(name="psum1", bufs=2, space="PSUM") as ps1:
                for a in range(SLAB):
                    Zt = Zt0 if a % 2 == 0 else Zt1
                    for c in range(CH):
                        nc.vector.tensor_scalar_mul(
                            Zt[:, c * D1 : c * D1 + D],
                            slabT[:, (a * CH + c) * D : (a * CH + c + 1) * D],
                            winv[:, a * CH + c : a * CH + c + 1],
                        )
                    pcv = ps1.tile([128, 132], F32, tag="pcv")
                    # fused [C | v] accumulation: rhs carries a ones column
                    for c in range(CH):
                        nc.tensor.matmul(
                            pcv[:, 0:129],
                            lhsT=Zt[:, c * D1 : c * D1 + D],
                            rhs=Zt[:, c * D1 : c * D1 + D + 1],
                            start=(c == 0), stop=(c == CH - 1),
                        )
                    scr2 = work.tile([128, D], BF16, tag="scr")
                    nc.scalar.activation(
                        scr2[:], pcv[:, 0:128], AF.Square,
                        accum_out=F2b[:, a : a + 1],
                    )
                    nc.scalar.activation(
                        v2b[:, a : a + 1], pcv[:, 128:129], AF.Square
                    )

            # ---- partial sums -> AllReduce ----
            red2 = stat.tile([128, 2], F32, tag="red2")
            nc.vector.tensor_reduce(
                red2[:, 0:1], v2b[:], axis=mybir.AxisListType.X, op=ALU.add
            )
            nc.vector.tensor_reduce(
                red2[:, 1:2], F2b[:], axis=mybir.AxisListType.X, op=ALU.add
            )
            arin = stat.tile([1, 8], F32, tag="arin")
            arout = stat.tile([1, 8], F32, tag="arout")
            nc.vector.memset(arin[:], 0.0)
            with tc.tile_pool(name="psumS", bufs=1, space="PSUM") as psS:
                pR = psS.tile([1, 2], F32, tag="pR")
                nc.tensor.matmul(
                    pR[:], lhsT=onesf[:], rhs=red2[:], start=True, stop=True
                )
                nc.vector.tensor_copy(arin[0:1, 0:2], pR[:])
            arin_d = dram.tile([1, 8], F32, tag="arin_d")
            arout_d = dram.tile([1, 8], F32, tag="arout_d")
            nc.gpsimd.dma_start(arin_d[:], arin[:])
            nc.gpsimd.collective_compute(
                "AllReduce",
                ALU.add,
                replica_groups=[list(range(NCORES))],
                ins=[arin_d.opt()],
                outs=[arout_d.opt()],
            )
            nc.gpsimd.dma_start(arout[:], arout_d[:])
            nc.vector.tensor_copy(outsb[0:1, O_AR : O_AR + 2], arin[0:1, 0:2])
            nc.vector.tensor_copy(outsb[0:1, O_AR + 2 : O_AR + 4], arout[0:1, 0:2])

            # ---- scalars (tiny [1,1] ops) ----
            # t1 cols: 0:T1 1:T2 2:s1 3:s2 4:1/cnt 5:mu 6:negmu 7:c2g
            t1 = stat.tile([1, 8], F32, tag="t1")
            cntA = sclT[0:1, 0:1]
            k1A = sclT[0:1, 1:2]
            nc.vector.tensor_scalar(
                out=t1[:, 0:1], in0=arout[0:1, 0:1], scalar1=k1A, scalar2=None,
                op0=ALU.subtract,
            )
            nc.vector.tensor_scalar(
                out=t1[:, 1:2], in0=arout[0:1, 1:2], scalar1=k1A, scalar2=None,
                op0=ALU.subtract,
            )
            nc.vector.scalar_tensor_tensor(
                out=t1[:, 2:3], in0=cntA, scalar=-PD_EPS, in1=t1[:, 0:1],
                op0=ALU.mult, op1=ALU.add,
            )
            tmp = stat.tile([1, 1], F32, tag="tmp")
            nc.vector.scalar_tensor_tensor(
                out=tmp[:], in0=t1[:, 0:1], scalar=-2.0 * PD_EPS,
                in1=t1[:, 1:2], op0=ALU.mult, op1=ALU.add,
            )
            nc.vector.scalar_tensor_tensor(
                out=t1[:, 3:4], in0=cntA, scalar=PD_EPS * PD_EPS, in1=tmp[:],
                op0=ALU.mult, op1=ALU.add,
            )
            nc.vector.reciprocal(t1[:, 4:5], cntA)
            nc.vector.tensor_tensor(t1[:, 5:6], t1[:, 2:3], t1[:, 4:5], op=ALU.mult)
            nc.vector.tensor_scalar_mul(t1[:, 6:7], t1[:, 5:6], -1.0)
            nc.vector.tensor_tensor(tmp[:], t1[:, 2:3], t1[:, 5:6], op=ALU.mult)
            nc.vector.tensor_sub(t1[:, 7:8], t1[:, 3:4], tmp[:])
            # scalrow = [cnt, c2g, negmu, 0] -> broadcast to 128 partitions
            scalrow = stat.tile([1, 4], F32, tag="scalrow")
            nc.vector.memset(scalrow[:], 0.0)
            nc.vector.tensor_copy(scalrow[:, 0:1], cntA)
            nc.vector.tensor_copy(scalrow[:, 1:2], t1[:, 7:8])
            nc.vector.tensor_copy(scalrow[:, 2:3], t1[:, 6:7])
            scalB = stat.tile([128, 4], F32, tag="scalB")
            with tc.tile_pool(name="psumB", bufs=1, space="PSUM") as psB:
                pB = psB.tile([128, 4], F32, tag="pB")
                nc.tensor.matmul(
                    pB[:], lhsT=ones1[:], rhs=scalrow[:], start=True, stop=True
                )
                nc.vector.tensor_copy(scalB[:], pB[:])
            cntB = scalB[:, 0:1]
            c2gB = scalB[:, 1:2]
            negmuB = scalB[:, 2:3]

            # debug scalars: s1, s2, 1/cnt, mu, negmu
            nc.vector.tensor_copy(outsb[0:1, O_DBG : O_DBG + 5], t1[:, 2:7])

            # ---- LUT/rounding probes through the exact main-path chain ----
            const01 = stat.tile([1, 2], F32, tag="const01")
            nc.vector.memset(const01[:, 0:1], 0.0)
            nc.vector.memset(const01[:, 1:2], 1.0)
            u2p = stat.tile([1, 2], BF16, tag="u2p")
            nc.scalar.activation(
                u2p[:], const01[:], AF.Square, bias=scalB[0:1, 2:3], scale=1.0
            )
            dpb = stat.tile([1, 2], BF16, tag="dpb")
            nc.scalar.activation(
                dpb[:], u2p[:], AF.Sqrt,
                bias=scalB[0:1, 1:2], scale=scalB[0:1, 0:1],
            )
            nc.vector.tensor_copy(outsb[0:1, O_D0A : O_D0A + 2], dpb[:])

            # ================= phase 2: feat =================
            load_slab(fea)
            for a in range(SLAB):
                norms_anchor(a)
            finish_winv()

            with tc.tile_pool(name="psum2", bufs=2, space="PSUM") as ps2, \
                 tc.tile_pool(name="psumR", bufs=1, space="PSUM") as psR:
                prow = psR.tile([1, DIAGW], F32, tag="prow")
                first_red = [True]
                for a0 in range(0, SLAB, 2):
                    u2 = work.tile([128, 2 * UW], BF16, tag="u2")
                    for h in range(2):
                        a = a0 + h
                        Fb = work.tile([128, CH * D], BF16, tag="Fb")
                        nc.vector.tensor_copy(Fb[:], aseg(a))
                        dgw = work.tile([128, CH * D], BF16, tag="dgw")
                        for c in range(CH):
                            nc.vector.tensor_scalar_mul(
                                dgw[:, c * D : (c + 1) * D], eyeb[:],
                                winv[:, a * CH + c : a * CH + c + 1],
                            )
                        psT = ps2.tile([128, CH * D], F32, tag="psT")
                        for c in range(CH):
                            nc.tensor.matmul(
                                psT[:, c * D : (c + 1) * D],
                                lhsT=Fb[:, c * D : (c + 1) * D],
                                rhs=dgw[:, c * D : (c + 1) * D],
                                start=True, stop=True,
                            )
                        ZfT = work.tile([128, CH * D], BF16, tag="ZT")
                        nc.any.tensor_copy(ZfT[:], psT[:])

                        pd = ps2.tile([128, DIAGW], F32, tag="pd")
                        po = ps2.tile([128, max(OFFW, 1)], F32, tag="po")
                        off = 0
                        for i in range(CH):
                            lh = ZfT[:, i * D : (i + 1) * D]
                            nc.tensor.matmul(
                                pd[:, i * D : (i + 1) * D], lhsT=lh, rhs=lh,
                                start=True, stop=True,
                            )
                            if i < CH - 1:
                                w = (CH - 1 - i) * D
                                nc.tensor.matmul(
                                    po[:, off : off + w], lhsT=lh,
                                    rhs=ZfT[:, (i + 1) * D : CH * D],
                                    start=True, stop=True,
                                )
                                off += w
                        # u2 = (x - mu)^2, both regions on ACT
                        base = UW * h
                        nc.scalar.activation(
                            u2[:, base : base + DIAGW], pd[:], AF.Square,
                            bias=negmuB, scale=1.0,
                        )
                        if OFFW:
                            nc.scalar.activation(
                                u2[:, base + DIAGW : base + UW], po[:, 0:OFFW],
                                AF.Square, bias=negmuB, scale=1.0,
                            )
                    # d = sqrt(cnt*u2 + c2g) for both anchors in one pass
                    dbuf = work.tile([128, 2 * UW], BF16, tag="dbuf")
                    nc.scalar.activation(
                        dbuf[:], u2[:], AF.Sqrt, bias=c2gB, scale=cntB
                    )
                    # PE accumulates weighted column sums (diag*1, off*2) in
                    # one long accumulation group
                    for h in range(2):
                        base = UW * h
                        last = a0 + 2 >= SLAB and h == 1
                        nc.tensor.matmul(
                            prow[:, 0:DIAGW], lhsT=onesb[:],
                            rhs=dbuf[:, base : base + DIAGW],
                            start=first_red[0], stop=(last and not OFFW),
                        )
                        first_red[0] = False
                        if OFFW:
                            nc.tensor.matmul(
                                prow[:, 0:OFFW], lhsT=twosb[:],
                                rhs=dbuf[:, base + DIAGW : base + UW],
                                start=False, stop=last,
                            )
                nc.vector.tensor_copy(outsb[0:1, O_DSUM : O_DSUM + DIAGW], prow[:])

            nc.sync.dma_start(out, outsb[:])

    nc.compile()
    nc._angleloss_NR = NR
    return nc


def _get_nc(NR):
    key = ("nc", NR)
    if key not in _CACHE:
        _CACHE[key] = _build(NR)
    return _CACHE[key]


def _host_prep(feat, true, pm):
    pm2 = pm & ~np.eye(N, dtype=bool)
    k = pm2.sum(axis=1).astype(np.int64)
    K1 = int(k.sum())
    cnt = int((k * k - k).sum())
    maxk = int(k.max()) if N else 0
    NR = 128 * int(np.ceil(max(maxk, 1) / 128.0))
    NR = max(NR, 128)
    CH = NR // 128

    # compact valid rows per anchor (order irrelevant), zero-pad to NR
    feag = np.zeros((N, NR, D), dtype=np.float32)
    trug = np.zeros((N, NR, D), dtype=np.float32)
    wmask = np.zeros((N, NR), dtype=np.float32)
    for a in range(N):
        idx = np.flatnonzero(pm2[a])
        ka = len(idx)
        feag[a, :ka] = feat[a, idx]
        trug[a, :ka] = true[a, idx]
        wmask[a, :ka] = 1.0

    scl = np.array([[cnt, K1]], dtype=np.float32)
    eye = np.eye(128, dtype=np.float32)
    in_maps = []
    for core in range(NCORES):
        g0 = core * SLAB
        wmk = np.ascontiguousarray(
            wmask[g0 : g0 + SLAB].reshape(SLAB * CH, 128).T
        )
        in_maps.append(
            {
                "tru": trug[g0 : g0 + SLAB],
                "fea": feag[g0 : g0 + SLAB],
                "wmk": wmk,
                "scl": scl,
                "eye": eye,
            }
        )
    return in_maps, cnt, K1, NR


def _combine(results, cnt, K1, NR):
    outs = [np.asarray(r["out"], dtype=np.float64)[0] for r in results]
    G = sum(o[O_DSUM : O_DSUM + 768].sum() for o in outs)
    d0 = outs[0][O_D0A]
    d1 = outs[0][O_D1]
    inv0 = float(N) * NR * NR - cnt - K1
    Sd = G - inv0 * d0 - K1 * d1
    return np.float32(0.5 * Sd / max(cnt, 1.0))


def kernel(feat_angle_dist_matrix, positive_masks, true_angle_dist_matrix):
    feat = np.ascontiguousarray(feat_angle_dist_matrix, dtype=np.float32)
    true = np.ascontiguousarray(true_angle_dist_matrix, dtype=np.float32)
    pm = np.asarray(positive_masks).astype(bool)

    in_maps, cnt, K1, NR = _host_prep(feat, true, pm)
    if cnt == 0:
        return np.float32(0.0)

    nc = _get_nc(NR)
    res = bass_utils.run_bass_kernel_spmd(nc, in_maps, core_ids=list(range(NCORES)))
    return _combine(res.results, cnt, K1, NR)



# revision 7
# speedup vs baseline: 1.4272x; 1.4272x over previous
"""AngleLossV2 distributed Bass kernel for 8 TRN2 NeuronCores.

Math (reference):
  mask[a,p,q] = pm[a,p] & pm[a,q] & (a!=p) & (a!=q) & (p!=q)
  fn = l2norm(feat, -1); tn = l2norm(true, -1)
  f[a,p,q] = <fn[a,p], fn[a,q]>;  t likewise
  cnt = sum(mask); tp = where(mask, t-eps, 0); s1 = sum(tp); s2 = sum(tp*tp)
  d = sqrt(max(cnt*f^2 - 2*f*s1 + s2, 0))
  loss = 0.5 * sum(where(mask, d, 0)) / max(cnt, 1)

Key algebra (per anchor a, over masked normalized rows z_p):
  sum_{p!=q valid} t   = ||sum_p z_p||^2 - k_a
  sum_{p!=q valid} t^2 = ||Z^T Z||_F^2 - k_a      (Z^T Z is [128,128])
  cnt = sum_a (k_a^2 - k_a), K1 = sum_a k_a       (host, exact)
  d(x) = sqrt(cnt*(x-mu)^2 + c2g),  mu = s1/cnt, c2g = s2 - s1^2/cnt

Device layout: the host compacts each anchor's valid rows, l2-normalizes
them exactly (f32, matching the reference), casts to bf16 and ships TWO
layouts: Zt row-major [rows, D+1] (ones-validity column fused for the
v = Z^T 1 sum) for the phase-1 D x D Gram C = Z^T [Z|1], and ZfT d-major
[D, rows] for phase-2 row-Gram blocks straight off the PE (no on-chip
normalization, casts or transposes).  Anchors are sorted by overflow
c1 = k-128 and snake-dealt over the 8 cores so slot s has a shared
ragged width w[s] (SPMD: one program for all cores).

Phase 2 per slot: g00 = Z0 Z0^T [128,128], g11 = Z1 Z1^T [128,w],
g01 = Z0 Z1^T [128,w] (weight 2).  Diag blocks (g00,g11) pack two slots
per PSUM bank, one Square per bank; off blocks stream into their own
banks.  The x2 off weight is folded into the Sqrt constants
(sqrt(4cnt*u + 4c2g) = 2d), so the d-sum is a plain Vector tensor_reduce
over everything.  Probes d0/d1/e0 run x=0/1 through the exact same
instruction chain so LUT and bf16 rounding bias cancels.  One AllReduce
of 2 scalars (S1/S2 parts) overlaps the phase-2 matmuls.  Host combines
per-core partials in float64.
"""

import sys
import numpy as np

for _p in ("/opt/trn_rl_repo",):
    if _p not in sys.path:
        sys.path.insert(0, _p)

import ml_dtypes

from concourse import bacc, bass, mybir, tile
from concourse import bass_utils

F32 = mybir.dt.float32
BF16 = mybir.dt.bfloat16
AF = mybir.ActivationFunctionType
ALU = mybir.AluOpType

N = 384
D = 128
NCORES = 8
SLAB = N // NCORES  # 48 anchor slots per core
NR = 256
E1 = D + 1  # z chunk + validity/ones column
NORM_EPS = 1e-6
PD_EPS = 1e-6
BF = ml_dtypes.bfloat16

# out row layout ([1, NOUT])
O_DSUM = 0
O_D0 = 1  # diag-chain probe at x=0
O_D1 = 2  # diag-chain probe at x=1
O_E0 = 3  # off-chain probe at x=0 (represents 2*d0 chain)
O_AR = 4  # arin0, arin1, arout0, arout1
O_DBG = 8  # s1, s2, mu, c2g
NOUT = 16

_CACHE = {}


def _build(wslots, cnt, K1):
    """wslots: tuple of 48 ragged chunk-1 widths (multiples of 8, <=128)."""
    cntf = float(cnt)

    nc = bacc.Bacc(
        "TRN2",
        target_bir_lowering=False,
        debug=False,
        num_devices=NCORES,
    )
    ztd_t = nc.dram_tensor("ztd", [SLAB, NR, E1], BF16, kind="ExternalInput")
    zfd_t = nc.dram_tensor("zfd", [SLAB, D, NR], BF16, kind="ExternalInput")
    cst_t = nc.dram_tensor("cst", [1, 4], F32, kind="ExternalInput")
    out_t = nc.dram_tensor("out", [1, NOUT], F32, kind="ExternalOutput")

    ztd = ztd_t.ap()
    zfd = zfd_t.ap()
    cst = cst_t.ap()
    out = out_t.ap()

    # diag stream: per slot 128 + w cols; off stream: w cols
    dwid = [128 + w for w in wslots]
    TOTD = sum(dwid)
    TOTO = sum(wslots)

    with tile.TileContext(nc) as tc:
        with (
            tc.tile_pool(name="stat", bufs=1) as stat,
            tc.tile_pool(name="work", bufs=2) as work,
            tc.tile_pool(name="dram", bufs=1, space="DRAM") as dram,
        ):
            ztb = stat.tile([128, SLAB * 2 * E1], BF16, tag="ztb")
            zfb = stat.tile([128, SLAB * NR], BF16, tag="zfb")
            u2d = stat.tile([128, TOTD], BF16, tag="u2d")
            u2o = stat.tile([128, max(TOTO, 2)], BF16, tag="u2o")
            dbd = stat.tile([128, TOTD], BF16, tag="dbd")
            dbo = stat.tile([128, max(TOTO, 2)], BF16, tag="dbo")
            F2b = stat.tile([128, SLAB], F32, tag="F2b")
            vall = stat.tile([128, SLAB], F32, tag="vall")
            scr = stat.tile([128, 129], BF16, tag="scr")  # STT dump
            redc = stat.tile([128, 32], F32, tag="redc")
            onesf = stat.tile([128, 1], F32, tag="onesf")
            ones1 = stat.tile([1, 128], F32, tag="ones1")
            cstT = stat.tile([1, 4], F32, tag="cstT")
            outsb = stat.tile([1, NOUT], F32, tag="outsb")
            arin = stat.tile([1, 8], F32, tag="arin")
            arout = stat.tile([1, 8], F32, tag="arout")
            t1 = stat.tile([1, 8], F32, tag="t1")
            scalrow = stat.tile([1, 4], F32, tag="scalrow")
            scalB = stat.tile([128, 4], F32, tag="scalB")
            const01 = stat.tile([1, 2], F32, tag="const01")
            u2p = stat.tile([1, 2], BF16, tag="u2p")

            nc.vector.memset(onesf[:], 1.0)
            nc.vector.memset(ones1[:], 1.0)
            nc.vector.memset(outsb[:], 0.0)
            nc.vector.memset(arin[:], 0.0)
            nc.vector.memset(const01[:, 0:1], 0.0)
            nc.vector.memset(const01[:, 1:2], 1.0)
            nc.gpsimd.dma_start(cstT[:], cst)

            # ---- input loads: zt on sync queue, zf split vector/gpsimd ----
            GRP = 4
            for gi, s0 in enumerate(range(0, SLAB, GRP)):
                dst = ztb[:, s0 * 2 * E1 : (s0 + GRP) * 2 * E1]
                nc.sync.dma_start(
                    dst.rearrange("p (b c e) -> p b c e", b=GRP, c=2),
                    ztd[s0 : s0 + GRP].rearrange("b (c p) e -> p b c e", p=128),
                )

            def load_zf(gi):
                s0 = gi * GRP
                dst = zfb[:, s0 * NR : (s0 + GRP) * NR]
                eng = nc.scalar if gi % 2 == 0 else nc.gpsimd
                eng.dma_start(
                    dst.rearrange("p (b r) -> p b r", b=GRP),
                    zfd[s0 : s0 + GRP].rearrange("b d r -> d b r"),
                )

            # ================= phase 1: true stats =================
            with tc.tile_pool(name="ps1", bufs=3, space="PSUM") as ps1:
                for s in range(SLAB):
                    if s % GRP == 0:
                        load_zf(s // GRP)
                    w = wslots[s]
                    off = s * 2 * E1
                    pcv = ps1.tile([128, E1], F32, tag="pcv")
                    nc.tensor.matmul(
                        pcv[:],
                        lhsT=ztb[:, off : off + D],
                        rhs=ztb[:, off : off + E1],
                        start=True,
                        stop=(w == 0),
                    )
                    if w:
                        nc.tensor.matmul(
                            pcv[:],
                            lhsT=ztb[0:w, off + E1 : off + E1 + D],
                            rhs=ztb[0:w, off + E1 : off + E1 + E1],
                            start=False,
                            stop=True,
                        )
                    if s % 2 == 0:
                        nc.scalar.activation(
                            scr[:, 0:D], pcv[:, 0:D], AF.Square,
                            accum_out=F2b[:, s : s + 1],
                        )
                    else:
                        scrb = work.tile([128, D], BF16, tag="scrb")
                        scrv = work.tile([128, D], BF16, tag="scrv")
                        nc.vector.tensor_copy(scrb[:], pcv[:, 0:D])
                        nc.vector.scalar_tensor_tensor(
                            out=scrv[:, 0:D],
                            in0=scrb[:],
                            scalar=1.0,
                            in1=scrb[:],
                            op0=ALU.mult,
                            op1=ALU.mult,
                            accum_out=F2b[:, s : s + 1],
                        )
                    nc.vector.tensor_copy(vall[:, s : s + 1], pcv[:, D : D + 1])

            # ---- partial sums -> AllReduce ----
            red2 = stat.tile([128, 2], F32, tag="red2")
            nc.vector.scalar_tensor_tensor(
                out=scr[:, 0:SLAB],
                in0=vall[:],
                scalar=1.0,
                in1=vall[:],
                op0=ALU.mult,
                op1=ALU.mult,
                accum_out=red2[:, 0:1],
            )
            nc.vector.tensor_reduce(
                red2[:, 1:2], F2b[:], axis=mybir.AxisListType.X, op=ALU.add
            )
            with tc.tile_pool(name="psS", bufs=1, space="PSUM") as psS:
                pR = psS.tile([1, 2], F32, tag="pR")
                nc.tensor.matmul(
                    pR[:], lhsT=onesf[:], rhs=red2[:], start=True, stop=True
                )
                nc.vector.tensor_copy(arin[0:1, 0:2], pR[:])
            arin_d = dram.tile([1, 8], F32, tag="arin_d")
            arout_d = dram.tile([1, 8], F32, tag="arout_d")
            nc.gpsimd.dma_start(arin_d[:], arin[:])
            nc.gpsimd.collective_compute(
                "AllReduce",
                ALU.add,
                replica_groups=[list(range(NCORES))],
                ins=[arin_d.opt()],
                outs=[arout_d.opt()],
            )
            nc.gpsimd.dma_start(arout[:], arout_d[:])
            nc.vector.tensor_copy(outsb[0:1, O_AR : O_AR + 2], arin[0:1, 0:2])
            nc.vector.tensor_copy(outsb[0:1, O_AR + 2 : O_AR + 4], arout[0:1, 0:2])

            # ---- scalars: s1 = AR0 + A ; s2 = (-2eps)*AR0 + AR1 + B2 ----
            # cst cols: 0:A 1:B2 2:invcnt 3:unused
            # t1 cols: 0:s1 1:s2 2:mu 3:negmu 4:c2g 5:c2g4 6:tmp
            nc.vector.tensor_scalar(
                out=t1[:, 0:1], in0=arout[0:1, 0:1], scalar1=cstT[0:1, 0:1],
                scalar2=None, op0=ALU.add,
            )
            nc.vector.tensor_scalar(
                out=t1[:, 6:7], in0=arout[0:1, 1:2], scalar1=cstT[0:1, 1:2],
                scalar2=None, op0=ALU.add,
            )
            nc.vector.scalar_tensor_tensor(
                out=t1[:, 1:2], in0=arout[0:1, 0:1], scalar=-2.0 * PD_EPS,
                in1=t1[:, 6:7], op0=ALU.mult, op1=ALU.add,
            )
            nc.vector.tensor_scalar(
                out=t1[:, 2:3], in0=t1[:, 0:1], scalar1=cstT[0:1, 2:3],
                scalar2=None, op0=ALU.mult,
            )
            nc.vector.tensor_scalar(
                out=t1[:, 3:4], in0=t1[:, 2:3], scalar1=-1.0, scalar2=None,
                op0=ALU.mult,
            )
            nc.vector.scalar_tensor_tensor(
                out=t1[:, 4:5], in0=t1[:, 3:4], scalar=t1[0:1, 0:1],
                in1=t1[:, 1:2], op0=ALU.mult, op1=ALU.add,
            )
            nc.vector.tensor_scalar(
                out=t1[:, 5:6], in0=t1[:, 4:5], scalar1=4.0, scalar2=None,
                op0=ALU.mult,
            )
            nc.vector.tensor_copy(outsb[0:1, O_DBG : O_DBG + 2], t1[:, 0:2])
            nc.vector.tensor_copy(outsb[0:1, O_DBG + 2 : O_DBG + 3], t1[:, 2:3])
            nc.vector.tensor_copy(outsb[0:1, O_DBG + 3 : O_DBG + 4], t1[:, 4:5])
            # scalrow = [negmu, c2g, c2g4, 0] -> broadcast to 128 partitions
            nc.vector.memset(scalrow[:], 0.0)
            nc.vector.tensor_copy(scalrow[:, 0:1], t1[:, 3:4])
            nc.vector.tensor_copy(scalrow[:, 1:2], t1[:, 4:5])
            nc.vector.tensor_copy(scalrow[:, 2:3], t1[:, 5:6])
            with tc.tile_pool(name="psB", bufs=1, space="PSUM") as psB:
                pB = psB.tile([128, 4], F32, tag="pB")
                nc.tensor.matmul(
                    pB[:], lhsT=ones1[:], rhs=scalrow[:], start=True, stop=True
                )
                nc.vector.tensor_copy(scalB[:], pB[:])
            negmuB = scalB[:, 0:1]
            c2gB = scalB[:, 1:2]
            c2g4B = scalB[:, 2:3]

            # ---- LUT/rounding probes through the exact main-path chain ----
            nc.scalar.activation(
                u2p[:], const01[:], AF.Square, bias=scalB[0:1, 0:1], scale=1.0
            )
            dpd = stat.tile([1, 2], BF16, tag="dpd")
            dpo = stat.tile([1, 1], BF16, tag="dpo")
            nc.scalar.activation(
                dpd[:], u2p[:], AF.Sqrt, bias=scalB[0:1, 1:2], scale=cntf
            )
            nc.scalar.activation(
                dpo[:], u2p[:, 0:1], AF.Sqrt, bias=scalB[0:1, 2:3],
                scale=4.0 * cntf,
            )
            nc.vector.tensor_copy(outsb[0:1, O_D0 : O_D0 + 2], dpd[:])
            nc.vector.tensor_copy(outsb[0:1, O_E0 : O_E0 + 1], dpo[:])

            # ================= phase 2: feat Gram + d =================
            # diag stream: pairs of slots share one PSUM bank
            # off stream: greedy-packed banks
            dcur = 0  # u2d/dbd write cursor
            ocur = 0  # u2o/dbo cursor
            d_sq = []  # (start, width) of emitted diag Square spans
            o_sq = []
            with (
                tc.tile_pool(name="psA", bufs=3, space="PSUM") as psA,
                tc.tile_pool(name="psO", bufs=2, space="PSUM") as psO,
            ):
                pa = None
                pa_used = 0
                po = None
                po_used = 0
                for s in range(SLAB):
                    w = wslots[s]
                    b = s * NR
                    if pa is None:
                        pa = psA.tile([128, 512], F32, tag="pa")
                        pa_used = 0
                    nc.tensor.matmul(
                        pa[:, pa_used : pa_used + 128],
                        lhsT=zfb[:, b : b + 128],
                        rhs=zfb[:, b : b + 128],
                        start=True, stop=True,
                    )
                    if w:
                        nc.tensor.matmul(
                            pa[:, pa_used + 128 : pa_used + 128 + w],
                            lhsT=zfb[:, b + 128 : b + 256],
                            rhs=zfb[:, b + 128 : b + 128 + w],
                            start=True, stop=True,
                        )
                    pa_used += 128 + w
                    if s % 2 == 1 or s == SLAB - 1:
                        nc.scalar.activation(
                            u2d[:, dcur : dcur + pa_used], pa[:, 0:pa_used],
                            AF.Square, bias=negmuB, scale=1.0,
                        )
                        d_sq.append((dcur, pa_used))
                        dcur += pa_used
                        pa = None
                    if w:
                        if po is not None and po_used + w > 512:
                            nc.scalar.activation(
                                u2o[:, ocur : ocur + po_used], po[:, 0:po_used],
                                AF.Square, bias=negmuB, scale=1.0,
                            )
                            o_sq.append((ocur, po_used))
                            ocur += po_used
                            po = None
                        if po is None:
                            po = psO.tile([128, 512], F32, tag="po")
                            po_used = 0
                        nc.tensor.matmul(
                            po[:, po_used : po_used + w],
                            lhsT=zfb[:, b : b + 128],
                            rhs=zfb[:, b + 128 : b + 128 + w],
                            start=True, stop=True,
                        )
                        po_used += w
                if po is not None and po_used:
                    nc.scalar.activation(
                        u2o[:, ocur : ocur + po_used], po[:, 0:po_used],
                        AF.Square, bias=negmuB, scale=1.0,
                    )
                    o_sq.append((ocur, po_used))
                    ocur += po_used

                # Sqrt chunks follow Square spans (2 spans per op), then
                # Vector reduce chunks behind them
                nred = 0

                def emit_sqrt(spans, u2t, dbt, scale, bias):
                    nonlocal nred
                    i = 0
                    while i < len(spans):
                        r0 = spans[i][0]
                        r1 = spans[min(i + 1, len(spans) - 1)]
                        r1 = r1[0] + r1[1]
                        nc.scalar.activation(
                            dbt[:, r0:r1], u2t[:, r0:r1], AF.Sqrt,
                            bias=bias, scale=scale,
                        )
                        nc.vector.tensor_reduce(
                            redc[:, nred : nred + 1], dbt[:, r0:r1],
                            axis=mybir.AxisListType.X, op=ALU.add,
                        )
                        nred += 1
                        i += 2

                emit_sqrt(d_sq, u2d, dbd, cntf, c2gB)
                emit_sqrt(o_sq, u2o, dbo, 4.0 * cntf, c2g4B)

            # ---- final d-sum ----
            redf = stat.tile([128, 1], F32, tag="redf")
            nc.vector.tensor_reduce(
                redf[:], redc[:, 0:max(nred, 1)], axis=mybir.AxisListType.X,
                op=ALU.add,
            )
            with tc.tile_pool(name="psF", bufs=1, space="PSUM") as psF:
                pF = psF.tile([1, 1], F32, tag="pF")
                nc.tensor.matmul(
                    pF[:], lhsT=onesf[:], rhs=redf[:], start=True, stop=True
                )
                nc.vector.tensor_copy(outsb[0:1, O_DSUM : O_DSUM + 1], pF[:])

            nc.sync.dma_start(out, outsb[:])

    nc.compile()
    return nc


def _get_nc(wslots, cnt, K1):
    key = ("nc", wslots)
    if key not in _CACHE:
        _CACHE[key] = _build(wslots, cnt, K1)
    return _CACHE[key]


def _host_prep(feat, true, pm):
    pm2 = pm & ~np.eye(N, dtype=bool)
    k = pm2.sum(axis=1).astype(np.int64)
    K1 = int(k.sum())
    cnt = int((k * k - k).sum())

    c0 = np.minimum(k, 128)
    c1 = np.maximum(k - 128, 0)
    assert int(k.max()) <= NR, "k exceeds 2 chunks"

    # sort anchors by c1 desc; slot s holds ranks [8s, 8s+8): shared width
    order = np.argsort(-c1, kind="stable")
    wslots = []
    for s in range(SLAB):
        m = int(c1[order[NCORES * s]])
        wslots.append(min(128, int(np.ceil(m / 8.0)) * 8) if m > 0 else 0)
    wslots = tuple(wslots)

    # normalize exactly like the reference (f32)
    def l2n(x):
        n = np.sqrt(np.sum(x.astype(np.float32) ** 2, axis=-1, keepdims=True))
        return (x / np.maximum(n, NORM_EPS)).astype(np.float32)

    fn = l2n(feat)
    tn = l2n(true)

    in_maps = []
    Zd = 0  # diag-region zero-value slots
    Zo = 0  # off-region zero-value slots (value = 2d chain)
    A = -(K1 + PD_EPS * cnt)
    B2 = -K1 + 2.0 * PD_EPS * K1 + PD_EPS * PD_EPS * cnt
    invcnt = 1.0 / max(cnt, 1)
    cst = np.array([[A, B2, invcnt, 0.0]], dtype=np.float32)
    for core in range(NCORES):
        zt = np.zeros((SLAB, NR, E1), dtype=BF)
        zf = np.zeros((SLAB, D, NR), dtype=BF)
        for s in range(SLAB):
            a = int(order[NCORES * s + core])
            idx = np.flatnonzero(pm2[a])
            ka = len(idx)
            w = wslots[s]
            if ka:
                zt[s, :ka, :D] = tn[a, idx]
                zt[s, :ka, D] = 1.0
                zf[s, :, :ka] = fn[a, idx].T
            a0 = int(c0[a])
            a1 = int(c1[a])
            Zd += (16384 + 128 * w) - (a0 * a0 + a1 * a1)
            Zo += 128 * w - a0 * a1
        in_maps.append({"ztd": zt, "zfd": zf, "cst": cst})
    return in_maps, cnt, K1, wslots, Zd, Zo


def _combine(results, cnt, K1, Zd, Zo):
    outs = [np.asarray(r["out"], dtype=np.float64)[0] for r in results]
    G = sum(o[O_DSUM] for o in outs)
    d0 = outs[0][O_D0]
    d1 = outs[0][O_D1]
    e0 = outs[0][O_E0]
    Sd = G - Zd * d0 - Zo * e0 - K1 * d1
    return np.float32(0.5 * Sd / max(cnt, 1.0))


def kernel(feat_angle_dist_matrix, positive_masks, true_angle_dist_matrix):
    feat = np.ascontiguousarray(feat_angle_dist_matrix, dtype=np.float32)
    true = np.ascontiguousarray(true_angle_dist_matrix, dtype=np.float32)
    pm = np.asarray(positive_masks).astype(bool)

    in_maps, cnt, K1, wslots, Zd, Zo = _host_prep(feat, true, pm)
    if cnt == 0:
        return np.float32(0.0)

    nc = _get_nc(wslots, cnt, K1)
    res = bass_utils.run_bass_kernel_spmd(nc, in_maps, core_ids=list(range(NCORES)))
    return _combine(res.results, cnt, K1, Zd, Zo)
